# revision 12
# baseline (speedup 1.0000x reference)
"""KANLinear2D Trainium2 kernel (8 NeuronCores, data-parallel over rows).

Math: out = silu(x) @ Wb.T + (sum_k B_spline_weight[:,k] * B3spline_k(x)) @ Ws.T

Spline identity: with t = (x - g0)/h and gamma = conv(w,[1,-4,6,-4,1])/6,
    g_i(t) = sum_{j=0..11} gamma[i,j] * relu(t - j)^3        (exact)
Channel 11 exists only to cancel the cubic tail for t > 11 (g == 0 there,
and g(11) == 0 by the same identity), so with tc = min(t, 11):
    g_i(t) = sum_{j=0..10} gamma[i,j] * relu(tc - j)^3       (exact)
The host ships pre-scaled fp16 t (for silu via act(scale=h, bias=g0... )) and
pre-clamped fp16 tc, so each DVE pass is one 6-ALU-op fused instruction
(the DVE datapath allows at most 8 ALU ops per instruction, so one cubic
channel per pass is the floor). relu(d)^3 == sq(d)*relu(d).
"""
import sys
import types

sys.path.insert(0, '/opt/trn_rl_repo')

import numpy as np

# ---------------------------------------------------------------------------
# Problem constants (hardcoded per contest contract)
B, E, IN, OUT = 256, 64, 512, 512
N_CORES = 8
TOTAL_ROWS = B * E            # 16384
ROWS = TOTAL_ROWS // N_CORES  # 2048 rows per core
GRID_SIZE, SPLINE_ORDER = 5, 3
H = (1.0 - (-1.0)) / GRID_SIZE          # 0.4
G0 = -1.0 - SPLINE_ORDER * H            # -2.2 (grid[0])
INV_H = 1.0 / H                         # 2.5
T_OFF = -G0 / H                         # +5.5 ; t = x*INV_H + T_OFF
N_CH = 11                               # truncated-power channels (ch 11 dropped via clamp)
FC = IN // 128                          # 4 feature chunks
BLOCK_SIZES = (640, 640, 512, 256)      # row blocks; small tail block hides
assert sum(BLOCK_SIZES) == ROWS         # the final matmul chain


def _gamma_from_w(w: np.ndarray) -> np.ndarray:
    """[IN, 8] spline weights -> [IN, 11] truncated-power coefficients."""
    from math import comb
    gamma = np.zeros((w.shape[0], N_CH), dtype=np.float64)
    for j in range(N_CH):
        for k in range(GRID_SIZE + SPLINE_ORDER):
            m = j - k
            if 0 <= m <= SPLINE_ORDER + 1:
                gamma[:, j] += w[:, k].astype(np.float64) * ((-1) ** m) * comb(4, m) / 6.0
    return gamma.astype(np.float32)


_CACHE = {}


def _register_dve_ops():
    from concourse.dve_spec import (
        Spec, Src0, Src1, C1, C2, relu, sq, lower, _has_src1 as has_src1,
    )
    from concourse.dve_uop import DveOpSpec
    from concourse import dve_ops
    from concourse.dve_ops import DveOp

    def reg(name, spec):
        for op in dve_ops.OPS:
            if op.name == name:
                return op
        row = dve_ops._CUSTOM_DVE_ROW_BASE + len(dve_ops.OPS)
        assert row < 0x20
        dve_ops._SUB_OPCODE_FOR_NAME[name] = row
        shas = {}
        for ver in ("v3", "v4"):
            uops = lower(spec, ver=ver)
            shas[ver] = DveOpSpec(name=name, opcode=row, uops=uops,
                                  rd1_en=has_src1(spec)).sha(ver)
        op = DveOp(name, spec, subdim=False, uops_sha=shas)
        dve_ops.OPS.append(op)
        dve_ops.CUSTOM_DVE_SPECS[name] = spec
        return op

    # channel j=0 on pre-clamped t: out = g0 * sq(t) * relu(t)   (4 ALU ops)
    first_op = reg("BSPL3_T0_ANT", Spec(
        body=C1 * (sq(Src0) * relu(Src0)),
        reference=lambda in0, s1: s1 * in0 * in0 * np.maximum(in0, 0)))
    # channel j (imm): out = acc + gj * sq(t-j) * relu(t-j)      (6 ALU ops)
    d = Src0 - C2
    acc_op = reg("BSPL3_TJ_ANT", Spec(
        body=Src1 + C1 * (sq(d) * relu(d)),
        reference=lambda in0, in1, s1, imm2: in1
        + s1 * ((in0 - imm2) ** 2 * np.maximum(in0 - imm2, 0))))
    return first_op, acc_op


def _install_axon_ntff_shim():
    """run_bass_kernel_spmd(trace=True) needs antenv.axon_hooks; provide it."""
    if 'antenv.axon_hooks' in sys.modules:
        return
    hook = None
    try:
        sys.path.insert(0, '/root/.axon_site/trn_agent_boot')
        from trn_boot import _ntff_profile_via_ctypes
        hook = _ntff_profile_via_ctypes('/opt/axon/libaxon_pjrt.so')
    except Exception:
        hook = None
    mod = types.ModuleType('antenv.axon_hooks')
    mod.get_axon_ntff_profile_hook = lambda: hook
    sys.modules['antenv.axon_hooks'] = mod


def _build_program():
    import concourse.bass as bass
    import concourse.tile as tile
    from concourse import bacc, mybir

    first_op, acc_op = _register_dve_ops()

    nc = bacc.Bacc("TRN2", target_bir_lowering=False, debug=False,
                   num_devices=N_CORES)
    f32 = mybir.dt.float32
    f16 = mybir.dt.float16
    bf16 = mybir.dt.bfloat16
    tT = nc.dram_tensor("tT", [IN, ROWS], f16, kind="ExternalInput").ap()
    tcT = nc.dram_tensor("tcT", [IN, ROWS], f16, kind="ExternalInput").ap()
    gamma_d = nc.dram_tensor("gamma", [IN, N_CH], f32, kind="ExternalInput").ap()
    bias_d = nc.dram_tensor("bias", [128, 1], f32, kind="ExternalInput").ap()
    wbt_d = nc.dram_tensor("wbt", [IN, OUT], bf16, kind="ExternalInput").ap()
    wst_d = nc.dram_tensor("wst", [IN, OUT], bf16, kind="ExternalInput").ap()
    out_d = nc.dram_tensor("out", [ROWS, OUT], bf16, kind="ExternalOutput").ap()

    with tile.TileContext(nc) as tc:
        with (
            tc.tile_pool(name="const", bufs=1) as const_pool,
            tc.tile_pool(name="chunks", bufs=1) as chunk_pool,
            tc.tile_pool(name="psum", bufs=8, space="PSUM") as psum_pool,
            tc.tile_pool(name="outb", bufs=4) as out_pool,
        ):
            # Persistent weight tiles, per feature chunk.
            bias_sb = const_pool.tile([128, 1], f32, tag="bias")
            nc.sync.dma_start(bias_sb[:], bias_d[:, :])
            wbt_sb, wst_sb, gam_sb = [], [], []
            for fc in range(FC):
                wb = const_pool.tile([128, OUT], bf16, tag=f"wbt{fc}")
                nc.sync.dma_start(wb[:], wbt_d[fc * 128:(fc + 1) * 128, :])
                ws = const_pool.tile([128, OUT], bf16, tag=f"wst{fc}")
                nc.sync.dma_start(ws[:], wst_d[fc * 128:(fc + 1) * 128, :])
                gm = const_pool.tile([128, N_CH], f32, tag=f"gam{fc}")
                nc.sync.dma_start(gm[:], gamma_d[fc * 128:(fc + 1) * 128, :])
                wbt_sb.append(wb)
                wst_sb.append(ws)
                gam_sb.append(gm)

            r0 = 0
            for blk, RB in enumerate(BLOCK_SIZES):
                silu_t, spl_t, acc_t, acc2_t, tc_t = [], [], [], [], []
                # DMA + silu per chunk
                for fc in range(FC):
                    tt = chunk_pool.tile([128, RB], f16, tag=f"tt{fc}_{blk}")
                    nc.sync.dma_start(tt[:], tT[fc * 128:(fc + 1) * 128,
                                                r0:r0 + RB])
                    tct = chunk_pool.tile([128, RB], f16, tag=f"tct{fc}_{blk}")
                    nc.sync.dma_start(tct[:], tcT[fc * 128:(fc + 1) * 128,
                                                  r0:r0 + RB])
                    sl = chunk_pool.tile([128, RB], bf16, tag=f"silu{fc}_{blk}")
                    nc.scalar.activation(sl[:], tt[:],
                                         mybir.ActivationFunctionType.Silu,
                                         bias=bias_sb[:, 0:1], scale=H)
                    silu_t.append(sl)
                    tc_t.append(tct)
                    ac = chunk_pool.tile([128, RB], f32, tag=f"acc{fc}_{blk}")
                    ac2 = chunk_pool.tile([128, RB], f32, tag=f"ac2{fc}_{blk}")
                    sp = chunk_pool.tile([128, RB], bf16, tag=f"spl{fc}_{blk}")
                    acc_t.append(ac)
                    acc2_t.append(ac2)
                    spl_t.append(sp)
                # DVE channel passes, interleaved across chunks so each
                # chain's drain hides under the other chains; accumulator
                # ping-pongs between two tiles to avoid same-address
                # read+write on every cycle.
                pp = [acc_t, acc2_t]
                for fc in range(FC):
                    g = gam_sb[fc]
                    nc.vector._custom_dve(first_op, out=acc_t[fc][:],
                                          in0=tc_t[fc][:], s1=g[:, 0:1])
                for j in range(1, N_CH - 1):
                    src = pp[(j - 1) % 2]
                    dst = pp[j % 2]
                    for fc in range(FC):
                        g = gam_sb[fc]
                        nc.vector._custom_dve(acc_op, out=dst[fc][:],
                                              in0=tc_t[fc][:], in1=src[fc][:],
                                              s1=g[:, j:j + 1], imm2=float(j))
                j = N_CH - 1
                src = pp[(j - 1) % 2]
                for fc in range(FC):
                    g = gam_sb[fc]
                    nc.vector._custom_dve(acc_op, out=spl_t[fc][:],
                                          in0=tc_t[fc][:], in1=src[fc][:],
                                          s1=g[:, j:j + 1], imm2=float(j))

                for rt in range(RB // 128):
                    ps = psum_pool.tile([128, OUT], f32, tag="ps")
                    for fc in range(FC):
                        nc.tensor.matmul(
                            ps[:],
                            lhsT=silu_t[fc][:, rt * 128:(rt + 1) * 128],
                            rhs=wbt_sb[fc][:],
                            start=(fc == 0), stop=False)
                    for fc in range(FC):
                        nc.tensor.matmul(
                            ps[:],
                            lhsT=spl_t[fc][:, rt * 128:(rt + 1) * 128],
                            rhs=wst_sb[fc][:],
                            start=False, stop=(fc == FC - 1))
                    ot = out_pool.tile([128, OUT], bf16, tag="ot")
                    nc.scalar.copy(ot[:], ps[:])
                    nc.sync.dma_start(
                        out_d[r0 + rt * 128:r0 + (rt + 1) * 128, :], ot[:])
                r0 += RB

    nc.compile()
    return nc


def _get_program():
    if "nc" not in _CACHE:
        _install_axon_ntff_shim()
        _CACHE["nc"] = _build_program()
    return _CACHE["nc"]


def _prep_inputs(x, base_weight, spline_weight, B_spline_weight):
    import ml_dtypes
    x = np.asarray(x, dtype=np.float32).reshape(TOTAL_ROWS, IN)
    t = (x * INV_H + T_OFF)
    tc = np.clip(t, 0.0, 11.0).astype(np.float16)
    t = t.astype(np.float16)
    gamma = _gamma_from_w(np.asarray(B_spline_weight, dtype=np.float32))
    wbt = np.ascontiguousarray(
        np.asarray(base_weight, np.float32).T.astype(ml_dtypes.bfloat16))
    wst = np.ascontiguousarray(
        np.asarray(spline_weight, np.float32).T.astype(ml_dtypes.bfloat16))
    in_maps = []
    for c in range(N_CORES):
        sl = slice(c * ROWS, (c + 1) * ROWS)
        in_maps.append({
            "tT": np.ascontiguousarray(t[sl].T),
            "tcT": np.ascontiguousarray(tc[sl].T),
            "gamma": gamma,
            "bias": np.full((128, 1), G0, dtype=np.float32),
            "wbt": wbt,
            "wst": wst,
        })
    return in_maps


def run(x, base_weight, spline_weight, B_spline_weight, trace=False,
        trace_kwargs=None):
    """Build+run; returns (output, BassKernelResults)."""
    from concourse.bass_utils import run_bass_kernel_spmd
    from concourse import bass_utils
    bass_utils.upload_artifacts = lambda tmpdir: str(tmpdir)

    nc = _get_program()
    in_maps = _prep_inputs(x, base_weight, spline_weight, B_spline_weight)
    res = run_bass_kernel_spmd(nc, in_maps, list(range(N_CORES)),
                               trace=trace, **(trace_kwargs or {}))
    out = np.concatenate([res.results[c]["out"] for c in range(N_CORES)],
                         axis=0).astype(np.float32).reshape(B, E, OUT)
    return out, res


def kernel(x, base_weight, spline_weight, B_spline_weight):
    out, _ = run(x, base_weight, spline_weight, B_spline_weight, trace=False)
    return out


# revision 15
# speedup vs baseline: 1.1265x; 1.1265x over previous
"""KANLinear2D Trainium2 kernel (8 NeuronCores, data-parallel over rows).

Math: out = silu(x) @ Wb.T + (sum_k B_spline_weight[:,k] * B3spline_k(x)) @ Ws.T

Spline identity: with t = (x - g0)/h and gamma = conv(w,[1,-4,6,-4,1])/6,
    g_i(t) = sum_{j=0..11} gamma[i,j] * relu(t - j)^3        (exact)
Channel 11 exists only to cancel the cubic tail for t > 11 (g == 0 there,
and g(11) == 0 by the same identity), so with tc = min(t, 11):
    g_i(t) = sum_{j=0..10} gamma[i,j] * relu(tc - j)^3       (exact)
The host ships pre-scaled fp16 t (for silu via act(scale=h, bias=g0... )) and
pre-clamped fp16 tc, so each DVE pass is one 6-ALU-op fused instruction
(the DVE datapath allows at most 8 ALU ops per instruction, so one cubic
channel per pass is the floor). relu(d)^3 == sq(d)*relu(d).
"""
import sys
import types

sys.path.insert(0, '/opt/trn_rl_repo')

import numpy as np

# ---------------------------------------------------------------------------
# Problem constants (hardcoded per contest contract)
B, E, IN, OUT = 256, 64, 512, 512
N_CORES = 8
TOTAL_ROWS = B * E            # 16384
ROWS = TOTAL_ROWS // N_CORES  # 2048 rows per core
GRID_SIZE, SPLINE_ORDER = 5, 3
H = (1.0 - (-1.0)) / GRID_SIZE          # 0.4
G0 = -1.0 - SPLINE_ORDER * H            # -2.2 (grid[0])
INV_H = 1.0 / H                         # 2.5
T_OFF = -G0 / H                         # +5.5 ; t = x*INV_H + T_OFF
N_CH = 11                               # truncated-power channels (ch 11 dropped via clamp)
FC = IN // 128                          # 4 feature chunks
BLOCK_SIZES = (1024, 640, 384)          # row blocks: few long DVE instructions
assert sum(BLOCK_SIZES) == ROWS         # (per-instruction bubble ~217ns), with
                                        # a tapering tail so matmuls hide


def _gamma_from_w(w: np.ndarray) -> np.ndarray:
    """[IN, 8] spline weights -> [IN, 11] truncated-power coefficients."""
    from math import comb
    gamma = np.zeros((w.shape[0], N_CH), dtype=np.float64)
    for j in range(N_CH):
        for k in range(GRID_SIZE + SPLINE_ORDER):
            m = j - k
            if 0 <= m <= SPLINE_ORDER + 1:
                gamma[:, j] += w[:, k].astype(np.float64) * ((-1) ** m) * comb(4, m) / 6.0
    return gamma.astype(np.float32)


_CACHE = {}


def _register_dve_ops():
    from concourse.dve_spec import (
        Spec, Src0, Src1, C1, C2, relu, sq, lower, _has_src1 as has_src1,
    )
    from concourse.dve_uop import DveOpSpec
    from concourse import dve_ops
    from concourse.dve_ops import DveOp

    def reg(name, spec):
        for op in dve_ops.OPS:
            if op.name == name:
                return op
        row = dve_ops._CUSTOM_DVE_ROW_BASE + len(dve_ops.OPS)
        assert row < 0x20
        dve_ops._SUB_OPCODE_FOR_NAME[name] = row
        shas = {}
        for ver in ("v3", "v4"):
            uops = lower(spec, ver=ver)
            shas[ver] = DveOpSpec(name=name, opcode=row, uops=uops,
                                  rd1_en=has_src1(spec)).sha(ver)
        op = DveOp(name, spec, subdim=False, uops_sha=shas)
        dve_ops.OPS.append(op)
        dve_ops.CUSTOM_DVE_SPECS[name] = spec
        return op

    # channel j=0 on pre-clamped t: out = g0 * sq(t) * relu(t)   (4 ALU ops)
    first_op = reg("BSPL3_T0_ANT", Spec(
        body=C1 * (sq(Src0) * relu(Src0)),
        reference=lambda in0, s1: s1 * in0 * in0 * np.maximum(in0, 0)))
    # channel j (imm): out = acc + gj * sq(t-j) * relu(t-j)      (6 ALU ops)
    d = Src0 - C2
    acc_op = reg("BSPL3_TJ_ANT", Spec(
        body=Src1 + C1 * (sq(d) * relu(d)),
        reference=lambda in0, in1, s1, imm2: in1
        + s1 * ((in0 - imm2) ** 2 * np.maximum(in0 - imm2, 0))))
    return first_op, acc_op


def _install_axon_ntff_shim():
    """run_bass_kernel_spmd(trace=True) needs antenv.axon_hooks; provide it."""
    if 'antenv.axon_hooks' in sys.modules:
        return
    hook = None
    try:
        sys.path.insert(0, '/root/.axon_site/trn_agent_boot')
        from trn_boot import _ntff_profile_via_ctypes
        hook = _ntff_profile_via_ctypes('/opt/axon/libaxon_pjrt.so')
    except Exception:
        hook = None
    mod = types.ModuleType('antenv.axon_hooks')
    mod.get_axon_ntff_profile_hook = lambda: hook
    sys.modules['antenv.axon_hooks'] = mod


def _build_program():
    import concourse.bass as bass
    import concourse.tile as tile
    from concourse import bacc, mybir

    first_op, acc_op = _register_dve_ops()

    nc = bacc.Bacc("TRN2", target_bir_lowering=False, debug=False,
                   num_devices=N_CORES)
    f32 = mybir.dt.float32
    f16 = mybir.dt.float16
    bf16 = mybir.dt.bfloat16
    tT = nc.dram_tensor("tT", [IN, ROWS], f16, kind="ExternalInput").ap()
    tcT = nc.dram_tensor("tcT", [IN, ROWS], f16, kind="ExternalInput").ap()
    gamma_d = nc.dram_tensor("gamma", [IN, N_CH], f32, kind="ExternalInput").ap()
    bias_d = nc.dram_tensor("bias", [128, 1], f32, kind="ExternalInput").ap()
    wbt_d = nc.dram_tensor("wbt", [IN, OUT], bf16, kind="ExternalInput").ap()
    wst_d = nc.dram_tensor("wst", [IN, OUT], bf16, kind="ExternalInput").ap()
    out_d = nc.dram_tensor("out", [ROWS, OUT], bf16, kind="ExternalOutput").ap()

    with tile.TileContext(nc) as tc:
        with (
            tc.tile_pool(name="const", bufs=1) as const_pool,
            tc.tile_pool(name="chunks", bufs=1) as chunk_pool,
            tc.tile_pool(name="psum", bufs=8, space="PSUM") as psum_pool,
            tc.tile_pool(name="outb", bufs=4) as out_pool,
        ):
            # First block's spline inputs + gammas go out on the DMA queue
            # before the (larger) weight tiles: the first DVE op can then
            # start ~4us earlier while weights stream during block 0.
            RB0 = BLOCK_SIZES[0]
            tct0, gam_sb = [], []
            for fc in range(FC):
                tct = chunk_pool.tile([128, RB0], f16, tag=f"tct{fc}_0")
                nc.sync.dma_start(tct[:], tcT[fc * 128:(fc + 1) * 128, 0:RB0])
                gm = const_pool.tile([128, N_CH], f32, tag=f"gam{fc}")
                nc.sync.dma_start(gm[:], gamma_d[fc * 128:(fc + 1) * 128, :])
                tct0.append(tct)
                gam_sb.append(gm)
            bias_sb = const_pool.tile([128, 1], f32, tag="bias")
            nc.sync.dma_start(bias_sb[:], bias_d[:, :])
            wbt_sb, wst_sb = [], []
            for fc in range(FC):
                wb = const_pool.tile([128, OUT], bf16, tag=f"wbt{fc}")
                nc.sync.dma_start(wb[:], wbt_d[fc * 128:(fc + 1) * 128, :])
                ws = const_pool.tile([128, OUT], bf16, tag=f"wst{fc}")
                nc.sync.dma_start(ws[:], wst_d[fc * 128:(fc + 1) * 128, :])
                wbt_sb.append(wb)
                wst_sb.append(ws)

            r0 = 0
            for blk, RB in enumerate(BLOCK_SIZES):
                silu_t, spl_t, acc_t, acc2_t, tc_t = [], [], [], [], []
                # DMA + silu per chunk
                for fc in range(FC):
                    tt = chunk_pool.tile([128, RB], f16, tag=f"tt{fc}_{blk}")
                    nc.sync.dma_start(tt[:], tT[fc * 128:(fc + 1) * 128,
                                                r0:r0 + RB])
                    if blk == 0:
                        tct = tct0[fc]
                    else:
                        tct = chunk_pool.tile([128, RB], f16,
                                              tag=f"tct{fc}_{blk}")
                        nc.sync.dma_start(tct[:], tcT[fc * 128:(fc + 1) * 128,
                                                      r0:r0 + RB])
                    sl = chunk_pool.tile([128, RB], bf16, tag=f"silu{fc}_{blk}")
                    nc.scalar.activation(sl[:], tt[:],
                                         mybir.ActivationFunctionType.Silu,
                                         bias=bias_sb[:, 0:1], scale=H)
                    silu_t.append(sl)
                    tc_t.append(tct)
                    ac = chunk_pool.tile([128, RB], f32, tag=f"acc{fc}_{blk}")
                    ac2 = chunk_pool.tile([128, RB], f32, tag=f"ac2{fc}_{blk}")
                    sp = chunk_pool.tile([128, RB], bf16, tag=f"spl{fc}_{blk}")
                    acc_t.append(ac)
                    acc2_t.append(ac2)
                    spl_t.append(sp)
                # DVE channel passes, interleaved across chunks so each
                # chain's drain hides under the other chains; accumulator
                # ping-pongs between two tiles to avoid same-address
                # read+write on every cycle.
                pp = [acc_t, acc2_t]
                for fc in range(FC):
                    g = gam_sb[fc]
                    nc.vector._custom_dve(first_op, out=acc_t[fc][:],
                                          in0=tc_t[fc][:], s1=g[:, 0:1])
                for j in range(1, N_CH - 1):
                    src = pp[(j - 1) % 2]
                    dst = pp[j % 2]
                    for fc in range(FC):
                        g = gam_sb[fc]
                        nc.vector._custom_dve(acc_op, out=dst[fc][:],
                                              in0=tc_t[fc][:], in1=src[fc][:],
                                              s1=g[:, j:j + 1], imm2=float(j))
                j = N_CH - 1
                src = pp[(j - 1) % 2]
                for fc in range(FC):
                    g = gam_sb[fc]
                    nc.vector._custom_dve(acc_op, out=spl_t[fc][:],
                                          in0=tc_t[fc][:], in1=src[fc][:],
                                          s1=g[:, j:j + 1], imm2=float(j))

                for rt in range(RB // 128):
                    ps = psum_pool.tile([128, OUT], f32, tag="ps")
                    for fc in range(FC):
                        nc.tensor.matmul(
                            ps[:],
                            lhsT=silu_t[fc][:, rt * 128:(rt + 1) * 128],
                            rhs=wbt_sb[fc][:],
                            start=(fc == 0), stop=False)
                    for fc in range(FC):
                        nc.tensor.matmul(
                            ps[:],
                            lhsT=spl_t[fc][:, rt * 128:(rt + 1) * 128],
                            rhs=wst_sb[fc][:],
                            start=False, stop=(fc == FC - 1))
                    ot = out_pool.tile([128, OUT], bf16, tag="ot")
                    nc.scalar.copy(ot[:], ps[:])
                    nc.sync.dma_start(
                        out_d[r0 + rt * 128:r0 + (rt + 1) * 128, :], ot[:])
                r0 += RB

    nc.compile()
    return nc


def _get_program():
    if "nc" not in _CACHE:
        _install_axon_ntff_shim()
        _CACHE["nc"] = _build_program()
    return _CACHE["nc"]


def _prep_inputs(x, base_weight, spline_weight, B_spline_weight):
    import ml_dtypes
    x = np.asarray(x, dtype=np.float32).reshape(TOTAL_ROWS, IN)
    t = (x * INV_H + T_OFF)
    tc = np.clip(t, 0.0, 11.0).astype(np.float16)
    t = t.astype(np.float16)
    gamma = _gamma_from_w(np.asarray(B_spline_weight, dtype=np.float32))
    wbt = np.ascontiguousarray(
        np.asarray(base_weight, np.float32).T.astype(ml_dtypes.bfloat16))
    wst = np.ascontiguousarray(
        np.asarray(spline_weight, np.float32).T.astype(ml_dtypes.bfloat16))
    in_maps = []
    for c in range(N_CORES):
        sl = slice(c * ROWS, (c + 1) * ROWS)
        in_maps.append({
            "tT": np.ascontiguousarray(t[sl].T),
            "tcT": np.ascontiguousarray(tc[sl].T),
            "gamma": gamma,
            "bias": np.full((128, 1), G0, dtype=np.float32),
            "wbt": wbt,
            "wst": wst,
        })
    return in_maps


def run(x, base_weight, spline_weight, B_spline_weight, trace=False,
        trace_kwargs=None):
    """Build+run; returns (output, BassKernelResults)."""
    from concourse.bass_utils import run_bass_kernel_spmd
    from concourse import bass_utils
    bass_utils.upload_artifacts = lambda tmpdir: str(tmpdir)

    nc = _get_program()
    in_maps = _prep_inputs(x, base_weight, spline_weight, B_spline_weight)
    res = run_bass_kernel_spmd(nc, in_maps, list(range(N_CORES)),
                               trace=trace, **(trace_kwargs or {}))
    out = np.concatenate([res.results[c]["out"] for c in range(N_CORES)],
                         axis=0).astype(np.float32).reshape(B, E, OUT)
    return out, res


def kernel(x, base_weight, spline_weight, B_spline_weight):
    out, _ = run(x, base_weight, spline_weight, B_spline_weight, trace=False)
    return out
